# revision 1
# baseline (speedup 1.0000x reference)
"""Batched GCN layer on 8 TRN2 NeuronCores.

Problem: out[b] = Dinv (A[b]+I) Dinv (X[b] @ W + b_vec), Dinv = diag(rowsum(A+I)^-1/2)
Shapes: B=8, N=4096, DIN=DOUT=64.  Sharding: one batch element per core.

Per-core algorithm (all compute on device):
  Host supplies AT = A[b].T (layout choice: lets the PE both row-sum A and run
  the aggregation matmul without any on-chip transpose of the 64 MB matrix),
  XTa = [X[b].T; ones], Wb = [W; b_vec], EYE = I_128.

  0. H[j] = XTa[:, j*128:(j+1)*128].T @ Wb          (PE, K=65) -> H (SBUF)
  1. d-pass: stream AT row-stripes [128, N]; ones.T @ stripe accumulates
     column sums of AT (= row sums of A) in PSUM. First `retain` stripes stay
     resident in SBUF.
  2. dinv = 1/sqrt(d+1) (ACT Sqrt + DVE recip); [128, N/128] column layout
     via DRAM bounce + PE transpose.
  3. G = dinv * H (row scale, DVE).
  4. matmul pass: outT[:, chunk] = sum_i G[i].T @ stripe_i[:, chunk]
     (+ G[j].T @ EYE for the +I term), PSUM-accumulated over all stripes;
     retained stripes come from SBUF, the rest are re-read.
  5. outT *= dinv (column scale), DMA out. Host transposes [64, N]->[N, 64].

The re-read is unavoidable: dinv depends on ALL row sums, so the scaled
matmul cannot consume A on the same read that computes the sums. Traffic is
reduced instead by the ship mode:

ship modes (what the host puts in DRAM for A^T):
  "f32"  : AT f32 (64 MB). d-pass + matmul in fp32 (optionally float32r —
           the single-pass fp32 PE mode, 4x plain fp32). Exact. ~2x128 MB.
  "hilo" : ATH, ATL bf16 with hi + lo == f32 value to ~2^-17 rel. d-pass
           reads hi only (32 MB); matmul accumulates hi@G + lo@G (64 MB).
           Near-f32 precision at 96 MB traffic, PE at full bf16 rate.
  "bf16" : ATH bf16 only (32 MB/pass, 64 MB total). ~1e-3 rel error.
"""

import numpy as np

B = 8
N = 4096
D = 64
P = 128
CHUNK = 512  # psum bank = 512 f32

_prog_cache = {}


def _patch_tile_drain():
    """This container's walrus cannot encode sync waits on InstDrain/InstNoOp
    with >1 wait ("Too many sync wait commands"). Split the end-of-TileContext
    global-clock waits across multiple sequencer NOPs, one proc each."""
    import concourse.tile as tile_mod
    from concourse.vector_clock import ScopedClock, VectorClock

    if getattr(tile_mod.TileContext, "_drain_patched", False):
        return

    def _drain_and_barrier(self, tick_clock, wait_clock):
        g = tick_clock.global_clock
        for p in range(64):
            try:
                tick = g.peek_next(p) - 1
            except Exception:
                break
            if tick <= 0:
                continue
            vc = VectorClock()
            vc.require_at_least(p, tick)
            nop_inst = self.nc.sync.nop(nofuse=True, hint=f"pre_drain_wait_{p}")
            wait_clock.add_sem_waits(nop_inst.ins, ScopedClock({None: vc}))
        self.nc.sync.drain()
        self.nc.all_engine_barrier()
        assert self.sems is not None
        popped = self.nc._tile_sem_poison_stack.pop()
        assert popped is self._sem_poison
        self.nc.clear_and_free_semaphores(list(self.sems.allocated().values()))
        self.nc.all_engine_barrier()

    tile_mod.TileContext._drain_and_barrier = _drain_and_barrier
    tile_mod.TileContext._drain_patched = True


def _split_multiwait(nc):
    """This container's walrus encodes at most ONE sync wait per instruction
    (and none on InstDrain) — 'Too many sync wait commands' otherwise. Tile
    emits multi-wait instructions freely, so after scheduling we peel excess
    waits onto fresh same-engine NOPs inserted immediately before the
    instruction. Per-engine streams execute in order, so an earlier wait on
    the same engine is equivalent."""
    from concourse import mybir

    cnt = 0
    for bb in nc.main_func.blocks:
        insts = bb.instructions
        out = []
        changed = False
        for ins in insts:
            si = ins.sync_info
            waits = list(si.on_wait) if si is not None else []
            limit = 0 if isinstance(ins, mybir.InstDrain) else 1
            if len(waits) > limit:
                keep = waits[-limit:] if limit else []
                for w in waits[:len(waits) - limit]:
                    cnt += 1
                    nop = mybir.InstNoOp(
                        name=f"I-wsplit-{cnt}", ins=[], outs=[])
                    nop.engine = ins.engine
                    nop.sync_info = mybir.SyncInfo(on_wait=[w], on_update=[])
                    out.append(nop)
                ins.sync_info = mybir.SyncInfo(
                    on_wait=keep, on_update=list(si.on_update))
                changed = True
            out.append(ins)
        if changed:
            bb.instructions = out
    return cnt


def build_program(n=N, reps=1, ship="bf16", f32r=True, retain=10, stripe_bufs=8,
                  split=True, trip=None):
    """Build the per-core bass program. Returns nc.

    trip: if set, wrap the body in a hardware For_i loop with that trip
    count (used for wall-clock timing: T(trip) - T(1) isolates device time
    from dispatch/transfer overhead)."""
    _patch_tile_drain()
    import concourse.bass as bass
    import concourse.tile as tile
    from concourse import mybir

    n_mb = n // P
    n_ch = (n + CHUNK - 1) // CHUNK
    assert n % P == 0 and n % CHUNK == 0
    assert 0 <= retain <= n_mb
    assert ship in ("f32", "hilo", "bf16")

    f32 = mybir.dt.float32
    bf16 = mybir.dt.bfloat16
    st_dt = f32 if ship == "f32" else bf16

    nc = bass.Bass(target_bir_lowering=False)
    if ship == "f32":
        ATH = nc.declare_dram_parameter("AT", [n, n], f32, isOutput=False)
        ATL = None
    else:
        ATH = nc.declare_dram_parameter("ATH", [n, n], bf16, isOutput=False)
        ATL = (nc.declare_dram_parameter("ATL", [n, n], bf16, isOutput=False)
               if ship == "hilo" else None)
    XTa = nc.declare_dram_parameter("XTa", [D + 1, n], f32, isOutput=False)
    Wb = nc.declare_dram_parameter("Wb", [D + 1, D], f32, isOutput=False)
    EYE = nc.declare_dram_parameter("EYE", [P, P], f32, isOutput=False)
    OT = nc.declare_dram_parameter("OT", [D, n], f32, isOutput=True)

    with tile.TileContext(nc) as tc:
        with tc.tile_pool(name="const", bufs=1) as cpool:
            xta_sb = cpool.tile([D + 1, n], f32)
            nc.sync.dma_start(xta_sb[:], XTa[:])
            wb_sb = cpool.tile([D + 1, D], f32)
            nc.sync.dma_start(wb_sb[:], Wb[:])
            eye_sb = cpool.tile([P, P], f32)
            nc.sync.dma_start(eye_sb[:], EYE[:])

            if trip is not None:
                with tc.For_i(0, trip, 1):
                    _one_rep(nc, tc, mybir, n, n_mb, n_ch, ship, f32r,
                             retain, stripe_bufs, ATH, ATL, OT,
                             xta_sb, wb_sb, eye_sb)
            else:
                for rep in range(reps):
                    _one_rep(nc, tc, mybir, n, n_mb, n_ch, ship, f32r,
                             retain, stripe_bufs, ATH, ATL, OT,
                             xta_sb, wb_sb, eye_sb)
    if split:
        _split_multiwait(nc)
    return nc


def _one_rep(nc, tc, mybir, n, n_mb, n_ch, ship, f32r, retain, stripe_bufs,
             ATH, ATL, OT, xta_sb, wb_sb, eye_sb):
    f32 = mybir.dt.float32
    f32r_dt = mybir.dt.float32r
    bf16 = mybir.dt.bfloat16
    st_dt = f32 if ship == "f32" else bf16

    def cast(ap):  # fast fp32 PE mode for the f32 ship mode
        return ap.bitcast(f32r_dt) if (ship == "f32" and f32r) else ap

    def load(pool, src, i, tag):
        st = pool.tile([P, n], st_dt, tag=tag, name=tag)
        nc.sync.dma_start(st[:], src[i * P:(i + 1) * P, :])
        return st

    with tc.tile_pool(name="work", bufs=1) as wpool, \
         tc.tile_pool(name="resident", bufs=1) as rpool, \
         tc.tile_pool(name="stripes", bufs=stripe_bufs) as spool:

        # ---- Phase 0: H = XTa.T @ Wb ----
        h_all = wpool.tile([P, n_mb, D], f32)
        with tc.tile_pool(name="hpsum", bufs=4, space="PSUM") as hpsum:
            for j in range(n_mb):
                hp = hpsum.tile([P, D], f32)
                nc.tensor.matmul(
                    hp[:], xta_sb[:, j * P:(j + 1) * P], wb_sb[:],
                    start=True, stop=True)
                nc.vector.tensor_copy(h_all[:, j, :], hp[:])

        ones_sb = wpool.tile([P, D], st_dt)
        nc.vector.memset(ones_sb[:], 1.0)
        out_sb = wpool.tile([D, n], f32)   # also sqrt scratch
        dinv_rep = wpool.tile([D, n], f32)

        # ---- Phase 1: d-pass over hi stripes ----
        residents = {}
        with tc.tile_pool(name="dpsum", bufs=1, space="PSUM") as dpsum:
            d_acc = [dpsum.tile([D, CHUNK], f32, name=f"d_acc{c}", tag=f"d_acc{c}")
                     for c in range(n_ch)]
            for i in range(n_mb):
                if i < retain:
                    st = load(rpool, ATH, i, f"res{i}")
                    residents[i] = st
                else:
                    st = load(spool, ATH, i, "stripe")
                for c in range(n_ch):
                    nc.tensor.matmul(
                        d_acc[c][:],
                        cast(ones_sb[:]),
                        cast(st[:, c * CHUNK:(c + 1) * CHUNK]),
                        start=(i == 0), stop=(i == n_mb - 1))
            for c in range(n_ch):
                nc.scalar.activation(
                    out_sb[:, c * CHUNK:(c + 1) * CHUNK], d_acc[c][:],
                    mybir.ActivationFunctionType.Sqrt, bias=1.0, scale=1.0)
            nc.vector.reciprocal(dinv_rep[:], out_sb[:])

        # ---- Phase 1b: dinv column layout via DRAM bounce + PE transpose ----
        # bounce dinv through row 0 of the OT output buffer (internal DRAM
        # tensors are not loadable under the PJRT runner); the final store
        # overwrites all of OT afterwards.
        dcol32 = wpool.tile([n_mb, P, 1], f32)
        nc.sync.dma_start(OT[0:1, :], dinv_rep[0:1, :])
        nc.sync.dma_start(
            dcol32[:, :, 0], OT[0, :].rearrange("(q p) -> q p", q=n_mb))
        dinv_col = wpool.tile([P, n_mb], f32)
        with tc.tile_pool(name="tpsum", bufs=1, space="PSUM") as tpsum:
            tp = tpsum.tile([P, n_mb], f32)
            nc.tensor.transpose(tp[:], dcol32[:, :, 0], eye_sb[0:n_mb, 0:n_mb])
            nc.vector.tensor_copy(dinv_col[:], tp[:])

        # ---- Phase 2: G = dinv * H (cast to the matmul dtype on write) ----
        g_mm = wpool.tile([P, n_mb, D], st_dt)
        nc.vector.tensor_tensor(
            g_mm[:], h_all[:],
            dinv_col[:, :, None].to_broadcast((P, n_mb, D)),
            mybir.AluOpType.mult)
        if st_dt == bf16:
            eye_mm = wpool.tile([P, P], bf16)
            nc.vector.tensor_copy(eye_mm[:], eye_sb[:])
        else:
            eye_mm = eye_sb

        # ---- Phase 3: outT accumulation ----
        with tc.tile_pool(name="opsum", bufs=1, space="PSUM") as opsum:
            o_acc = [opsum.tile([D, CHUNK], f32, name=f"o_acc{c}", tag=f"o_acc{c}")
                     for c in range(n_ch)]
            per_bank = CHUNK // P
            for j in range(n_mb):  # +I term; first touch per bank start=True
                c, k = divmod(j, per_bank)
                nc.tensor.matmul(
                    o_acc[c][:, k * P:(k + 1) * P],
                    cast(g_mm[:, j, :]), cast(eye_mm[:]),
                    start=(k == 0), stop=False)
            order = list(range(retain)) + list(range(retain, n_mb))
            for idx, i in enumerate(order):
                last = idx == n_mb - 1
                st = residents.get(i)
                if st is None:
                    st = load(spool, ATH, i, "stripe")
                lo = load(spool, ATL, i, "lostripe") if ATL is not None else None
                for c in range(n_ch):
                    sl = slice(c * CHUNK, (c + 1) * CHUNK)
                    nc.tensor.matmul(
                        o_acc[c][:], cast(g_mm[:, i, :]), cast(st[:, sl]),
                        start=False, stop=(last and lo is None))
                    if lo is not None:
                        nc.tensor.matmul(
                            o_acc[c][:], g_mm[:, i, :], lo[:, sl],
                            start=False, stop=last)
            # ---- Phase 4: final column scale + store ----
            for c in range(n_ch):
                sl = slice(c * CHUNK, (c + 1) * CHUNK)
                nc.vector.tensor_tensor(
                    out_sb[:, sl], o_acc[c][:], dinv_rep[:, sl],
                    mybir.AluOpType.mult)
        nc.sync.dma_start(OT[:], out_sb[:])


def _get_program(key):
    if key not in _prog_cache:
        n, reps, ship, f32r, retain = key
        _prog_cache[key] = build_program(
            n=n, reps=reps, ship=ship, f32r=f32r, retain=retain)
    return _prog_cache[key]


def make_in_maps(X, A, W, b, ship="bf16"):
    import ml_dtypes
    n = A.shape[1]
    eye = np.eye(P, dtype=np.float32)
    Wb = np.concatenate([W.astype(np.float32), b.astype(np.float32)[None, :]],
                        axis=0)
    in_maps = []
    for i in range(X.shape[0]):
        AT = np.ascontiguousarray(np.asarray(A[i]).T)
        XTa = np.concatenate(
            [np.ascontiguousarray(np.asarray(X[i]).T),
             np.ones((1, n), np.float32)], axis=0)
        m = {"XTa": XTa, "Wb": Wb, "EYE": eye}
        if ship == "f32":
            m["AT"] = AT
        else:
            hi = AT.astype(ml_dtypes.bfloat16)
            m["ATH"] = hi
            if ship == "hilo":
                m["ATL"] = (AT - hi.astype(np.float32)).astype(ml_dtypes.bfloat16)
        in_maps.append(m)
    return in_maps


def kernel(X, A, W, b, reps=1, ship="bf16", f32r=True, retain=10):
    from concourse.bass_utils import run_bass_kernel_spmd

    X = np.asarray(X, dtype=np.float32)
    A = np.asarray(A, dtype=np.float32)
    W = np.asarray(W, dtype=np.float32)
    b = np.asarray(b, dtype=np.float32)
    n_b, n, _ = A.shape
    nc = _get_program((n, reps, ship, f32r, retain))
    in_maps = make_in_maps(X, A, W, b, ship=ship)
    res = run_bass_kernel_spmd(nc, in_maps, list(range(n_b)))
    out = np.stack([res.results[i]["OT"].T for i in range(n_b)])
    return np.ascontiguousarray(out)



# revision 9
# speedup vs baseline: 1.7842x; 1.7842x over previous
"""Batched GCN layer on 8 TRN2 NeuronCores — single-pass fp8-resident design.

Problem: out[b] = Dinv (A[b]+I) Dinv (X[b] @ W + b_vec), Dinv = diag(rowsum(A+I)^-1/2)
Shapes: B=8, N=4096, DIN=DOUT=64.  Sharding: one batch element per core.

Key idea vs the previous 2-pass bf16 kernel (212us): A is uniform[0,1], so a
*centered* fp8 e3m4 encoding a8 = e3m4(4*(A.T - 0.5)) carries bf16-class
absolute error at HALF the bytes (16 MB/core).  That fits entirely in SBUF, so
A is read from HBM exactly once, and the aggregation matmul streams it from
SBUF.  Simulated end-to-end rel err (max-abs / max-scale): 7.6e-3 vs the 2e-2
gate.

Per-core timeline (all on device):
  0. H = XTa.T @ Wb on PE (bf16) while the first A8 stripes arrive.
  1. A8 stripes [128, N] DMA to SBUF (resident).  Column-sum (= degree) is
     accumulated per stripe as it lands: most stripes via PE ones-matmul into
     PSUM d_acc, every 3rd stripe via a DVE tensor_tensor add into a bf16
     accumulator (folded into d_acc at the end) so the d-pass finishes with
     the DMA instead of trailing it.
  2. d = colsum/4 + 2049 (the /4 undoes the x4 encoding; 2048 restores the
     +0.5 center; +1 is the +I term).  dinv = d^-1/2 twice:
       - compact [128, 32] layout for scaling H -> G: row 0 of d bounced
         through DRAM, PE-transposed, then sqrt+recip on 32-wide tiles
         (fast path; unblocks the matmul).
       - replicated [64, N] layout for the final column scale (sqrt from
         PSUM + in-place reciprocal; overlaps the matmul).
  3. g = bf16(dinv * H / 4).  Centering correction: out += 0.5*colsum(G) per
     output channel, a rank-1 term realized as one K=1 f32r matmul per chunk
     with a constant 2.0 row (corr = colsum of g via 4 ones-matmuls + DVE
     reduces).
  4. Aggregation, chunk-outer: for each 512-col chunk, accumulate 32 resident
     stripe matmuls (bf16 G stationary x e3m4 A moving - mixed dtype verified
     exact on HW), 4 diagonal-block matmuls vs 4*I bf16 (the +I term), and the
     correction matmul into one PSUM bank; DVE-scale by dinv and DMA out.
     Completed chunks stream out while later chunks compute.
"""

import numpy as np

B = 8
N = 4096
D = 64
P = 128
CHUNK = 512

_prog_cache = {}


def _patch_tile_drain():
    """This container's walrus cannot encode sync waits on InstDrain/InstNoOp
    with >1 wait ("Too many sync wait commands"). Split the end-of-TileContext
    global-clock waits across multiple sequencer NOPs, one proc each."""
    import concourse.tile as tile_mod
    from concourse.vector_clock import ScopedClock, VectorClock

    if getattr(tile_mod.TileContext, "_drain_patched", False):
        return

    def _drain_and_barrier(self, tick_clock, wait_clock):
        g = tick_clock.global_clock
        for p in range(64):
            try:
                tick = g.peek_next(p) - 1
            except Exception:
                break
            if tick <= 0:
                continue
            vc = VectorClock()
            vc.require_at_least(p, tick)
            nop_inst = self.nc.sync.nop(nofuse=True, hint=f"pre_drain_wait_{p}")
            wait_clock.add_sem_waits(nop_inst.ins, ScopedClock({None: vc}))
        self.nc.sync.drain()
        self.nc.all_engine_barrier()
        assert self.sems is not None
        popped = self.nc._tile_sem_poison_stack.pop()
        assert popped is self._sem_poison
        self.nc.clear_and_free_semaphores(list(self.sems.allocated().values()))
        self.nc.all_engine_barrier()

    tile_mod.TileContext._drain_and_barrier = _drain_and_barrier
    tile_mod.TileContext._drain_patched = True


def _split_multiwait(nc):
    """This container's walrus encodes at most ONE sync wait per instruction
    (and none on InstDrain) — 'Too many sync wait commands' otherwise. Tile
    emits multi-wait instructions freely, so after scheduling we peel excess
    waits onto fresh same-engine NOPs inserted immediately before the
    instruction. Per-engine streams execute in order, so an earlier wait on
    the same engine is equivalent."""
    from concourse import mybir

    cnt = 0
    for bb in nc.main_func.blocks:
        insts = bb.instructions
        out = []
        changed = False
        for ins in insts:
            si = ins.sync_info
            waits = list(si.on_wait) if si is not None else []
            limit = 0 if isinstance(ins, mybir.InstDrain) else 1
            if len(waits) > limit:
                keep = waits[-limit:] if limit else []
                for w in waits[:len(waits) - limit]:
                    cnt += 1
                    nop = mybir.InstNoOp(
                        name=f"I-wsplit-{cnt}", ins=[], outs=[])
                    nop.engine = ins.engine
                    nop.sync_info = mybir.SyncInfo(on_wait=[w], on_update=[])
                    out.append(nop)
                ins.sync_info = mybir.SyncInfo(
                    on_wait=keep, on_update=list(si.on_update))
                changed = True
            out.append(ins)
        if changed:
            bb.instructions = out
    return cnt


def build_program(n=N, reps=1, trip=None, dve_every=3, **_ignored):
    """Build the per-core bass program. Returns nc.

    trip: if set, wrap the body in a hardware For_i loop with that trip
    count (used for wall-clock timing: T(trip) - T(1) isolates device time
    from dispatch/transfer overhead). The full A8 load is inside the loop
    body, so per-iteration time includes the HBM read of A."""
    _patch_tile_drain()
    import concourse.bass as bass
    import concourse.tile as tile
    from concourse import mybir

    n_mb = n // P
    n_ch = (n + CHUNK - 1) // CHUNK
    assert n % P == 0 and n % CHUNK == 0

    f32 = mybir.dt.float32
    bf16 = mybir.dt.bfloat16
    e3 = mybir.dt.float8e3

    nc = bass.Bass(target_bir_lowering=False)
    A8 = nc.declare_dram_parameter("A8", [n, n], e3, isOutput=False)
    XTA = nc.declare_dram_parameter("XTA", [D + 1, n], bf16, isOutput=False)
    WB = nc.declare_dram_parameter("WB", [D + 1, D], bf16, isOutput=False)
    EYE = nc.declare_dram_parameter("EYE", [32, 32], f32, isOutput=False)
    EYE4 = nc.declare_dram_parameter("EYE4", [P, P], bf16, isOutput=False)
    OT = nc.declare_dram_parameter("OT", [D, n], f32, isOutput=True)

    with tile.TileContext(nc) as tc:
        with tc.tile_pool(name="const", bufs=1) as cpool:
            xta_sb = cpool.tile([D + 1, n], bf16)
            nc.sync.dma_start(xta_sb[:], XTA[:])
            wb_sb = cpool.tile([D + 1, D], bf16)
            nc.sync.dma_start(wb_sb[:], WB[:])
            eye_sb = cpool.tile([32, 32], f32)
            nc.sync.dma_start(eye_sb[:], EYE[:])
            eye4_sb = cpool.tile([P, P], bf16)
            nc.sync.dma_start(eye4_sb[:], EYE4[:])
            ones_bf = cpool.tile([P, D], bf16)
            nc.vector.memset(ones_bf[:], 1.0)
            ones2 = cpool.tile([1, CHUNK], bf16)
            nc.vector.memset(ones2[:], 2.0)
            bias_rep = cpool.tile([P, 1], f32)
            nc.vector.memset(bias_rep[:], 2049.0)
            bias_col = cpool.tile([P, 1], f32)
            nc.vector.memset(bias_col[:], 32784.0)

            args = (nc, tc, mybir, n, n_mb, n_ch, dve_every,
                    A8, OT, xta_sb, wb_sb, eye_sb, eye4_sb, ones_bf, ones2,
                    bias_rep, bias_col)
            if trip is not None:
                with tc.For_i(0, trip, 1):
                    _one_rep(*args)
            else:
                for _ in range(reps):
                    _one_rep(*args)
    _split_multiwait(nc)
    return nc


def _one_rep(nc, tc, mybir, n, n_mb, n_ch, dve_every,
             A8, OT, xta_sb, wb_sb, eye_sb, eye4_sb, ones_bf, ones2,
             bias_rep, bias_col):
    f32 = mybir.dt.float32
    f32r = mybir.dt.float32r
    bf16 = mybir.dt.bfloat16
    e3 = mybir.dt.float8e3
    Sqrt = mybir.ActivationFunctionType.Sqrt
    mult = mybir.AluOpType.mult
    add = mybir.AluOpType.add
    AX = mybir.AxisListType.X

    # every dve_every-th stripe (starting at 1) accumulates on DVE instead of
    # PE so the degree pass finishes together with the DMA stream
    dve_set = set(range(1, n_mb, dve_every)) if dve_every else set()

    with tc.tile_pool(name="a8", bufs=1) as apool, \
         tc.tile_pool(name="work", bufs=1) as wpool:

        stripes = []
        for i in range(n_mb):
            st = apool.tile([P, n], e3, name=f"a8_{i}", tag=f"a8_{i}")
            stripes.append(st)

        acc_v = wpool.tile([P, n], bf16)
        h_all = wpool.tile([P, n_mb, D], f32)
        g_q = wpool.tile([P, n_mb, D], bf16)
        d_row = wpool.tile([1, n], f32)
        dinv_rep = wpool.tile([D, n], f32)
        dcol32 = wpool.tile([n_mb, P], f32)
        dcol = wpool.tile([P, n_mb], f32)
        dsq_col = wpool.tile([P, n_mb], f32)
        dinv_colq = wpool.tile([P, n_mb], f32)
        corr4 = wpool.tile([1, 4, D], f32)
        corr = wpool.tile([1, D], f32)
        corr_hi = wpool.tile([1, D], bf16)
        corr_lo = wpool.tile([1, D], bf16)

        nc.vector.memset(acc_v[:], 0.0)

        # issue all stripe loads up-front, in order, on the sync ring
        for i in range(n_mb):
            nc.sync.dma_start(stripes[i][:], A8[i * P:(i + 1) * P, :])

        # ---- Phase 0: H = XTa.T @ Wb (PE, while stripes arrive) ----
        with tc.tile_pool(name="hpsum", bufs=2, space="PSUM") as hpsum:
            for blk in range(n_mb // 8):
                hp = hpsum.tile([P, 8 * D], f32, name=f"hp{blk}",
                                tag="hp")
                for jj in range(8):
                    j = blk * 8 + jj
                    nc.tensor.matmul(
                        hp[:, jj * D:(jj + 1) * D],
                        xta_sb[:, j * P:(j + 1) * P], wb_sb[:],
                        start=True, stop=True)
                nc.scalar.copy(h_all[:, blk * 8:(blk + 1) * 8, :], hp[:])

        # ---- Phase 1: degree pass over arriving stripes ----
        with tc.tile_pool(name="dpsum", bufs=1, space="PSUM") as dpsum:
            d_acc = [dpsum.tile([D, CHUNK], f32, name=f"d_acc{c}",
                                tag=f"d_acc{c}") for c in range(n_ch)]
            first_pe = True
            for i in range(n_mb):
                if i in dve_set:
                    nc.vector.tensor_tensor(
                        acc_v[:], acc_v[:], stripes[i][:], add)
                else:
                    for c in range(n_ch):
                        nc.tensor.matmul(
                            d_acc[c][:], ones_bf[:],
                            stripes[i][:, c * CHUNK:(c + 1) * CHUNK],
                            start=first_pe, stop=False)
                    first_pe = False
            # fold the DVE accumulator into PSUM
            for c in range(n_ch):
                nc.tensor.matmul(
                    d_acc[c][:], ones_bf[:],
                    acc_v[:, c * CHUNK:(c + 1) * CHUNK],
                    start=False, stop=True)
            # raw colsum row 0 -> SBUF (feeds the compact dinv path)
            for c in range(n_ch):
                nc.scalar.copy(d_row[0:1, c * CHUNK:(c + 1) * CHUNK],
                               d_acc[c][0:1, :])
            # replicated path: dinv_rep = (0.25*colsum + 2049)^-1/2
            # (sqrt now from PSUM; in-place reciprocal later, off the
            # critical path)
            for c in range(n_ch):
                nc.scalar.activation(
                    dinv_rep[:, c * CHUNK:(c + 1) * CHUNK], d_acc[c][:],
                    Sqrt, bias=bias_rep[0:D, :], scale=0.25)

        # ---- Phase 2: compact dinv via DRAM bounce + PE transpose ----
        # bounce raw colsum through row 0 of OT (overwritten by the final
        # store); gives [32,128] layout for a single PE transpose.
        nc.sync.dma_start(OT[0:1, :], d_row[0:1, :])
        nc.sync.dma_start(
            dcol32[:, :], OT[0, :].rearrange("(q p) -> q p", q=n_mb))
        with tc.tile_pool(name="tpsum", bufs=1, space="PSUM") as tpsum:
            tp = tpsum.tile([P, n_mb], f32)
            nc.tensor.transpose(tp[:], dcol32[:, :], eye_sb[:])
            nc.vector.tensor_copy(dcol[:], tp[:])
        # dinv_colq = (4*colsum + 32784)^-1/2 = 0.25 * dinv  (folds the /4
        # of the a8 encoding into G)
        nc.scalar.activation(dsq_col[:], dcol[:], Sqrt,
                             bias=bias_col[:], scale=4.0)
        nc.vector.reciprocal(dinv_colq[:], dsq_col[:])

        # ---- Phase 3: G = dinv/4 * H (bf16), centering correction ----
        nc.vector.tensor_tensor(
            g_q[:], h_all[:],
            dinv_colq[:, :, None].to_broadcast((P, n_mb, D)), mult)
        with tc.tile_pool(name="cpsum", bufs=1, space="PSUM") as cpsum:
            cs = [cpsum.tile([1, CHUNK], f32, name=f"cs{t}", tag=f"cs{t}")
                  for t in range(4)]
            for t in range(4):
                nc.tensor.matmul(
                    cs[t][:], ones_bf[:, 0:1], g_q[:, t * 8:(t + 1) * 8, :],
                    start=True, stop=True)
            for t in range(4):
                nc.vector.tensor_reduce(
                    corr4[0:1, t, :],
                    cs[t].rearrange("p (s d) -> p d s", s=8), AX, add)
        nc.vector.tensor_reduce(
            corr[0:1, :], corr4.rearrange("p t d -> p d t"), AX, add)
        nc.vector.tensor_copy(corr_hi[:], corr[:])
        nc.vector.tensor_tensor(corr_lo[:], corr[:], corr_hi[:],
                                mybir.AluOpType.subtract)
        # finish the replicated dinv (overlaps matmul below)
        nc.vector.reciprocal(dinv_rep[:], dinv_rep[:])

        # ---- Phase 4: aggregation, chunk-outer ----
        with tc.tile_pool(name="opsum", bufs=1, space="PSUM") as opsum, \
             tc.tile_pool(name="obuf", bufs=1) as obuf:
            for c in range(n_ch):
                sl = slice(c * CHUNK, (c + 1) * CHUNK)
                o_acc = opsum.tile([D, CHUNK], f32, name=f"o_acc{c}",
                                   tag=f"o_acc{c % 4}")
                for i in range(n_mb):
                    nc.tensor.matmul(
                        o_acc[:], g_q[:, i, :], stripes[i][:, sl],
                        start=(i == 0), stop=False)
                per_bank = CHUNK // P
                for jj in range(per_bank):
                    j = c * per_bank + jj
                    nc.tensor.matmul(
                        o_acc[:, jj * P:(jj + 1) * P], g_q[:, j, :],
                        eye4_sb[:], start=False, stop=False)
                nc.tensor.matmul(
                    o_acc[:], corr_hi[:], ones2[:],
                    start=False, stop=False)
                nc.tensor.matmul(
                    o_acc[:], corr_lo[:], ones2[:],
                    start=False, stop=True)
                ob = obuf.tile([D, CHUNK], f32, name=f"ob{c}",
                               tag=f"ob{c % 4}")
                nc.vector.tensor_tensor(ob[:], o_acc[:], dinv_rep[:, sl],
                                        mult)
                nc.sync.dma_start(OT[:, sl], ob[:])


def _get_program(key):
    if key not in _prog_cache:
        n, reps = key
        _prog_cache[key] = build_program(n=n, reps=reps)
    return _prog_cache[key]


def make_in_maps(X, A, W, b, **_ignored):
    import ml_dtypes
    n = A.shape[1]
    e3 = ml_dtypes.float8_e3m4
    bf = ml_dtypes.bfloat16
    eye = np.eye(32, dtype=np.float32)
    eye4 = (4.0 * np.eye(P, dtype=np.float32)).astype(bf)
    Wb = np.concatenate([W.astype(np.float32),
                         b.astype(np.float32)[None, :]], axis=0).astype(bf)
    in_maps = []
    for i in range(X.shape[0]):
        AT = np.ascontiguousarray(np.asarray(A[i]).T, dtype=np.float32)
        A8 = ((AT - 0.5) * 4.0).astype(e3)
        XTa = np.concatenate(
            [np.ascontiguousarray(np.asarray(X[i]).T),
             np.ones((1, n), np.float32)], axis=0).astype(bf)
        in_maps.append({"A8": A8, "XTA": XTa, "WB": Wb,
                        "EYE": eye, "EYE4": eye4})
    return in_maps


def kernel(X, A, W, b, reps=1, **_ignored):
    from concourse.bass_utils import run_bass_kernel_spmd

    X = np.asarray(X, dtype=np.float32)
    A = np.asarray(A, dtype=np.float32)
    W = np.asarray(W, dtype=np.float32)
    b = np.asarray(b, dtype=np.float32)
    n_b, n, _ = A.shape
    nc = _get_program((n, reps))
    in_maps = make_in_maps(X, A, W, b)
    res = run_bass_kernel_spmd(nc, in_maps, list(range(n_b)))
    out = np.stack([res.results[i]["OT"].T for i in range(n_b)])
    return np.ascontiguousarray(out)
